# revision 4
# baseline (speedup 1.0000x reference)
"""12-bit ripple-carry adder (SNN gate semantics) on 8 TRN2 NeuronCores.

Inputs A, B: (4194304, 12) float32 binary {0,1}, bit 11 = LSB.
Returns (sum_bits (4194304, 12) f32, carry_out (4194304, 1) f32) — matching
the reference's ripple-carry semantics exactly (all values are exact small
integers in f32, so the result is bit-exact).

Strategy (data parallel, no collectives):
 - Shard the batch across 8 cores (524288 rows each).
 - Per tile (128 partitions x F=512 rows/partition, 12 bits contiguous/row):
     T = A + B          computed by the DMA itself (SWDGE CCE accumulate)
     M = 2 - T          on the Scalar (ACT) engine
     carries            ONE vector-engine prefix scan, processed in reversed
                        free-dim order (LSB->MSB):  state' = (G*state) >= M
                        where G is 1 everywhere except 0 at each row's LSB,
                        which resets the carry chain at row boundaries.
     D = T + c_in       one tensor_tensor add (cols 0..10; col 11 has c_in=0)
     S = D - 2*c_out    one scalar_tensor_tensor op
 - DMA S and the carry column back out.
"""
import numpy as np
import concourse.tile as tile
from concourse import bacc, mybir
from concourse.bass_utils import run_bass_kernel_spmd

N_BITS = 12
BATCH = 4_194_304
N_CORES = 8
SHARD = BATCH // N_CORES        # 524288 rows per core
P = 128                         # SBUF partitions
F = 512                         # rows per partition per tile
ROWS_PER_TILE = P * F           # 65536
TILES = SHARD // ROWS_PER_TILE  # 8
W = N_BITS * F                  # free elems per partition per tile
SCAN_ROWS = 128                 # rows per scan instruction (128*12=1536 <= 2048)


def _build():
    nc = bacc.Bacc("TRN2", target_bir_lowering=False, debug=False,
                   num_devices=N_CORES)
    A = nc.dram_tensor("A", [SHARD, N_BITS], mybir.dt.float32, kind="ExternalInput")
    B = nc.dram_tensor("B", [SHARD, N_BITS], mybir.dt.float32, kind="ExternalInput")
    S = nc.dram_tensor("S", [SHARD, N_BITS], mybir.dt.float32, kind="ExternalOutput")
    C = nc.dram_tensor("C", [SHARD, 1], mybir.dt.float32, kind="ExternalOutput")

    with tile.TileContext(nc) as tc:
        with tc.tile_pool(name="const", bufs=1) as constp, \
             tc.tile_pool(name="work", bufs=2) as work:
            # Gate: 1.0 everywhere, 0.0 at col 11 (LSB) of each 12-col row.
            G = constp.tile([P, W], mybir.dt.float32)
            nc.vector.memset(G[:], 1.0)
            G3 = G[:].rearrange("p (f b) -> p f b", b=N_BITS)
            nc.vector.memset(G3[:, :, N_BITS - 1 : N_BITS], 0.0)

            for t in range(TILES):
                r0, r1 = t * ROWS_PER_TILE, (t + 1) * ROWS_PER_TILE
                Av = A[r0:r1, :].rearrange("(p f) b -> p (f b)", p=P)
                Bv = B[r0:r1, :].rearrange("(p f) b -> p (f b)", p=P)
                Sv = S[r0:r1, :].rearrange("(p f) b -> p (f b)", p=P)
                Cv = C[r0:r1, :].rearrange("(p f) b -> p (f b)", p=P)

                X = work.tile([P, W], mybir.dt.float32)   # A -> T=A+B -> D
                M = work.tile([P, W], mybir.dt.float32)   # 2-T, then reused for S
                Ct = work.tile([P, W], mybir.dt.float32)  # carry-out per bit
                co = work.tile([P, F], mybir.dt.float32)  # final carry per row

                nc.gpsimd.dma_start(out=X[:], in_=Av)
                # CCE-accumulate path mis-executes above ~2048 elems/partition
                # per instruction — split into row-aligned chunks.
                for c0 in range(0, F, SCAN_ROWS):
                    lo, hi = c0 * N_BITS, (c0 + SCAN_ROWS) * N_BITS
                    nc.gpsimd.dma_start(out=X[:][:, lo:hi], in_=Bv[:, lo:hi],
                                        accum_op=mybir.AluOpType.add)

                nc.scalar.activation(M[:], X[:],
                                     mybir.ActivationFunctionType.Copy,
                                     bias=2.0, scale=-1.0)

                # The DVE splits APs with any dim count > 2048, which would
                # reset the scan state mid-stream. Row-aligned chunks are
                # independent (the gate G zeroes the carry at every row
                # boundary), so emit one scan per SCAN_ROWS rows.
                for c0 in range(0, F, SCAN_ROWS):
                    lo, hi = c0 * N_BITS, (c0 + SCAN_ROWS) * N_BITS
                    nc.vector.tensor_tensor_scan(
                        out=Ct[:][:, lo:hi][:, ::-1],
                        data0=G[:][:, lo:hi][:, ::-1],
                        data1=M[:][:, lo:hi][:, ::-1], initial=0.0,
                        op0=mybir.AluOpType.mult, op1=mybir.AluOpType.is_ge,
                    )

                X3 = X[:].rearrange("p (f b) -> p f b", b=N_BITS)
                C3 = Ct[:].rearrange("p (f b) -> p f b", b=N_BITS)
                nc.vector.tensor_tensor(
                    out=X3[:, :, 0 : N_BITS - 1], in0=X3[:, :, 0 : N_BITS - 1],
                    in1=C3[:, :, 1:N_BITS], op=mybir.AluOpType.add,
                )
                # S = (Ct * -2) + D, written over M (dead after the scan)
                nc.vector.scalar_tensor_tensor(
                    out=M[:], in0=Ct[:], scalar=-2.0, in1=X[:],
                    op0=mybir.AluOpType.mult, op1=mybir.AluOpType.add,
                )
                nc.scalar.copy(co[:], C3[:, :, 0:1])

                nc.sync.dma_start(out=Sv, in_=M[:])
                nc.sync.dma_start(out=Cv, in_=co[:])
    nc.compile()
    return nc


_NC = None


def kernel(A: np.ndarray, B: np.ndarray):
    global _NC
    if _NC is None:
        _NC = _build()
    A = np.ascontiguousarray(A, dtype=np.float32)
    B = np.ascontiguousarray(B, dtype=np.float32)
    in_maps = [
        {"A": A[i * SHARD : (i + 1) * SHARD], "B": B[i * SHARD : (i + 1) * SHARD]}
        for i in range(N_CORES)
    ]
    res = run_bass_kernel_spmd(_NC, in_maps, core_ids=list(range(N_CORES)))
    S = np.concatenate([r["S"] for r in res.results], axis=0)
    C = np.concatenate([r["C"] for r in res.results], axis=0)
    return S, C


# revision 5
# speedup vs baseline: 1.1562x; 1.1562x over previous
"""12-bit ripple-carry adder (SNN gate semantics) on 8 TRN2 NeuronCores.

Inputs A, B: (4194304, 12) float32 binary {0,1}, bit 11 = LSB.
Returns (sum_bits (4194304, 12) f32, carry_out (4194304, 1) f32), bit-exact
vs the reference (all values are exact small integers in f32).

Strategy (data parallel, no collectives):
 - Shard the batch across 8 cores (524288 rows each).
 - Per tile (128 partitions x F rows/partition, 12 bits contiguous per row):
     T = A + B            on GPSIMD (frees DVE; DMA stays plain/line-rate)
     D_j = T_j + c_in_j   ONE vector-engine prefix scan in reversed free-dim
                          order (LSB->MSB):  state' = (G2 is_le state) + T
                          with G2 = 2.0 everywhere and 10.0 at each row's
                          LSB, which zeroes the carry at row boundaries.
     c = (D >= 2)         tensor_scalar (2x mode)     [c_out per bit]
     S = D - 2*c          one scalar_tensor_tensor
     carry_out            col 0 of c
 - All DMAs are plain HWDGE loads/stores at line rate.
"""
import numpy as np
import concourse.tile as tile
from concourse import bacc, mybir
from concourse.bass_utils import run_bass_kernel_spmd

N_BITS = 12
BATCH = 4_194_304
N_CORES = 8
SHARD = BATCH // N_CORES        # 524288 rows per core
P = 128                         # SBUF partitions
F = 256                         # rows per partition per tile
ROWS_PER_TILE = P * F           # 32768
TILES = SHARD // ROWS_PER_TILE  # 16
W = N_BITS * F                  # free elems per partition per tile
SCAN_ROWS = 128                 # rows per scan instruction (1536 elems <= 2048)
BUFS = 3


def _build():
    nc = bacc.Bacc("TRN2", target_bir_lowering=False, debug=False,
                   num_devices=N_CORES)
    A = nc.dram_tensor("A", [SHARD, N_BITS], mybir.dt.float32, kind="ExternalInput")
    B = nc.dram_tensor("B", [SHARD, N_BITS], mybir.dt.float32, kind="ExternalInput")
    S = nc.dram_tensor("S", [SHARD, N_BITS], mybir.dt.float32, kind="ExternalOutput")
    C = nc.dram_tensor("C", [SHARD, 1], mybir.dt.float32, kind="ExternalOutput")

    with tile.TileContext(nc) as tc:
        with tc.tile_pool(name="const", bufs=1) as constp, \
             tc.tile_pool(name="work", bufs=BUFS) as work:
            # Scan gate/threshold: 2.0 everywhere, 10.0 at col 11 (LSB).
            G2 = constp.tile([P, W], mybir.dt.float32)
            nc.vector.memset(G2[:], 2.0)
            G3 = G2[:].rearrange("p (f b) -> p f b", b=N_BITS)
            nc.vector.memset(G3[:, :, N_BITS - 1 : N_BITS], 10.0)

            for t in range(TILES):
                r0, r1 = t * ROWS_PER_TILE, (t + 1) * ROWS_PER_TILE
                Av = A[r0:r1, :].rearrange("(p f) b -> p (f b)", p=P)
                Bv = B[r0:r1, :].rearrange("(p f) b -> p (f b)", p=P)
                Sv = S[r0:r1, :].rearrange("(p f) b -> p (f b)", p=P)
                Cv = C[r0:r1, :].rearrange("(p f) b -> p (f b)", p=P)

                XA = work.tile([P, W], mybir.dt.float32)  # A, then T = A+B
                XB = work.tile([P, W], mybir.dt.float32)  # B, then S
                Dt = work.tile([P, W], mybir.dt.float32)  # D = T + c_in
                ct = work.tile([P, W], mybir.dt.float32)  # c_out per bit
                co = work.tile([P, F], mybir.dt.float32)  # final carry per row

                nc.sync.dma_start(out=XA[:], in_=Av)
                nc.sync.dma_start(out=XB[:], in_=Bv)

                nc.gpsimd.tensor_tensor(out=XA[:], in0=XA[:], in1=XB[:],
                                        op=mybir.AluOpType.add)

                # D-scan, reversed (LSB->MSB); chunks are row-aligned and
                # independent (G2's 10.0 kills the carry at each LSB), which
                # also keeps each instruction under the 2048-elem AP limit.
                for c0 in range(0, F, SCAN_ROWS):
                    lo, hi = c0 * N_BITS, (c0 + SCAN_ROWS) * N_BITS
                    nc.vector.tensor_tensor_scan(
                        out=Dt[:][:, lo:hi][:, ::-1],
                        data0=G2[:][:, lo:hi][:, ::-1],
                        data1=XA[:][:, lo:hi][:, ::-1], initial=0.0,
                        op0=mybir.AluOpType.is_le, op1=mybir.AluOpType.add,
                    )

                nc.vector.tensor_scalar(out=ct[:], in0=Dt[:], scalar1=2.0,
                                        scalar2=None, op0=mybir.AluOpType.is_ge)
                # S = (c * -2) + D
                nc.vector.scalar_tensor_tensor(
                    out=XB[:], in0=ct[:], scalar=-2.0, in1=Dt[:],
                    op0=mybir.AluOpType.mult, op1=mybir.AluOpType.add,
                )
                c3 = ct[:].rearrange("p (f b) -> p f b", b=N_BITS)
                nc.scalar.copy(co[:], c3[:, :, 0:1])

                nc.scalar.dma_start(out=Sv, in_=XB[:])
                nc.scalar.dma_start(out=Cv, in_=co[:])
    nc.compile()
    return nc


_NC = None


def kernel(A: np.ndarray, B: np.ndarray):
    global _NC
    if _NC is None:
        _NC = _build()
    A = np.ascontiguousarray(A, dtype=np.float32)
    B = np.ascontiguousarray(B, dtype=np.float32)
    in_maps = [
        {"A": A[i * SHARD : (i + 1) * SHARD], "B": B[i * SHARD : (i + 1) * SHARD]}
        for i in range(N_CORES)
    ]
    res = run_bass_kernel_spmd(_NC, in_maps, core_ids=list(range(N_CORES)))
    S = np.concatenate([r["S"] for r in res.results], axis=0)
    C = np.concatenate([r["C"] for r in res.results], axis=0)
    return S, C


# revision 8
# speedup vs baseline: 1.1855x; 1.0253x over previous
"""12-bit ripple-carry adder (SNN gate semantics) on 8 TRN2 NeuronCores.

Inputs A, B: (4194304, 12) float32 binary {0,1}, bit 11 = LSB.
Returns (sum_bits (4194304, 12) f32, carry_out (4194304, 1) f32), bit-exact
vs the reference (all values are exact small integers in f32).

Strategy (data parallel, no collectives). Per tile (128 partitions x F rows,
12 bits contiguous per row):
  T = A + B            on GPSIMD (keeps DVE free; DMAs stay plain/line-rate)
  D_j = T_j + c_in_j   ONE vector-engine prefix scan in reversed free-dim
                       order (LSB->MSB):  state' = (G2 is_le state) + T
                       with G2 = 2.0 everywhere, 10.0 at each row's LSB
                       (kills the carry at row boundaries, so row-aligned
                       scan chunks are independent).
  sg = Sign(D - 1.5)   on ACT; sg = 2*c_out - 1
  S = (D - 1) - sg     one scalar_tensor_tensor on DVE
  carry_out = 0.5*sg|col0 + 0.5   via ACT copy
All DMAs are plain HWDGE at line rate (loads on sync, stores on scalar).
"""
import numpy as np
import concourse.tile as tile
from concourse import bacc, mybir
from concourse.bass_utils import run_bass_kernel_spmd

N_BITS = 12
BATCH = 4_194_304
N_CORES = 8
SHARD = BATCH // N_CORES        # 524288 rows per core
P = 128                         # SBUF partitions
F = 256                         # rows per partition per tile
ROWS_PER_TILE = P * F           # 32768
TILES = SHARD // ROWS_PER_TILE  # 16
W = N_BITS * F                  # free elems per partition per tile
SCAN_ROWS = 128                 # rows per scan instruction (1536 elems <= 2048)
BUFS = 3


def _build():
    nc = bacc.Bacc("TRN2", target_bir_lowering=False, debug=False,
                   num_devices=N_CORES)
    A = nc.dram_tensor("A", [SHARD, N_BITS], mybir.dt.float32, kind="ExternalInput")
    B = nc.dram_tensor("B", [SHARD, N_BITS], mybir.dt.float32, kind="ExternalInput")
    S = nc.dram_tensor("S", [SHARD, N_BITS], mybir.dt.float32, kind="ExternalOutput")
    C = nc.dram_tensor("C", [SHARD, 1], mybir.dt.float32, kind="ExternalOutput")

    with tile.TileContext(nc) as tc:
        with tc.tile_pool(name="const", bufs=1) as constp, \
             tc.tile_pool(name="work", bufs=BUFS) as work:
            # Scan gate/threshold: 2.0 everywhere, 10.0 at col 11 (LSB).
            G2 = constp.tile([P, W], mybir.dt.float32)
            nc.vector.memset(G2[:], 2.0)
            G3 = G2[:].rearrange("p (f b) -> p f b", b=N_BITS)
            nc.vector.memset(G3[:, :, N_BITS - 1 : N_BITS], 10.0)
            bneg = constp.tile([P, 1], mybir.dt.float32)
            nc.vector.memset(bneg[:], -1.5)

            for t in range(TILES):
                r0, r1 = t * ROWS_PER_TILE, (t + 1) * ROWS_PER_TILE
                Av = A[r0:r1, :].rearrange("(p f) b -> p (f b)", p=P)
                Bv = B[r0:r1, :].rearrange("(p f) b -> p (f b)", p=P)
                Sv = S[r0:r1, :].rearrange("(p f) b -> p (f b)", p=P)
                Cv = C[r0:r1, :].rearrange("(p f) b -> p (f b)", p=P)

                XA = work.tile([P, W], mybir.dt.float32)  # A -> T -> S
                XB = work.tile([P, W], mybir.dt.float32)  # B -> D
                sg = work.tile([P, W], mybir.dt.float32)  # sign(D-1.5)
                co = work.tile([P, F], mybir.dt.float32)  # final carry per row

                nc.sync.dma_start(out=XA[:], in_=Av)
                nc.sync.dma_start(out=XB[:], in_=Bv)

                nc.gpsimd.tensor_tensor(out=XA[:], in0=XA[:], in1=XB[:],
                                        op=mybir.AluOpType.add)

                # D-scan, reversed (LSB->MSB), into XB (B is dead).
                for c0 in range(0, F, SCAN_ROWS):
                    lo, hi = c0 * N_BITS, (c0 + SCAN_ROWS) * N_BITS
                    nc.vector.tensor_tensor_scan(
                        out=XB[:][:, lo:hi][:, ::-1],
                        data0=G2[:][:, lo:hi][:, ::-1],
                        data1=XA[:][:, lo:hi][:, ::-1], initial=0.0,
                        op0=mybir.AluOpType.is_le, op1=mybir.AluOpType.add,
                    )

                # sg = sign(D - 1.5) = 2*c_out - 1  (exact: D-1.5 is never 0)
                nc.scalar.activation(sg[:], XB[:],
                                     mybir.ActivationFunctionType.Sign,
                                     bias=bneg[:], scale=1.0)
                # S = (D - 1) - sg, into XA (T is dead)
                nc.vector.scalar_tensor_tensor(
                    out=XA[:], in0=XB[:], scalar=-1.0, in1=sg[:],
                    op0=mybir.AluOpType.add, op1=mybir.AluOpType.subtract,
                )
                # carry = (sg|col0 + 1)/2
                s3 = sg[:].rearrange("p (f b) -> p f b", b=N_BITS)
                nc.scalar.activation(co[:], s3[:, :, 0:1],
                                     mybir.ActivationFunctionType.Copy,
                                     bias=0.5, scale=0.5)

                nc.scalar.dma_start(out=Sv, in_=XA[:])
                nc.scalar.dma_start(out=Cv, in_=co[:])
    nc.compile()
    return nc


_NC = None


def kernel(A: np.ndarray, B: np.ndarray):
    global _NC
    if _NC is None:
        _NC = _build()
    A = np.ascontiguousarray(A, dtype=np.float32)
    B = np.ascontiguousarray(B, dtype=np.float32)
    in_maps = [
        {"A": A[i * SHARD : (i + 1) * SHARD], "B": B[i * SHARD : (i + 1) * SHARD]}
        for i in range(N_CORES)
    ]
    res = run_bass_kernel_spmd(_NC, in_maps, core_ids=list(range(N_CORES)))
    S = np.concatenate([r["S"] for r in res.results], axis=0)
    C = np.concatenate([r["C"] for r in res.results], axis=0)
    return S, C


# revision 9
# speedup vs baseline: 1.1943x; 1.0074x over previous
"""12-bit ripple-carry adder (SNN gate semantics) on 8 TRN2 NeuronCores.

Inputs A, B: (4194304, 12) float32 binary {0,1}, bit 11 = LSB.
Returns (sum_bits (4194304, 12) f32, carry_out (4194304, 1) f32), bit-exact
vs the reference (all values are exact small integers in f32).

Data parallel, no collectives; radix-4 carry chain. Per tile (128 partitions
x F rows, 12 bits contiguous per row, 6 bit-pairs per row):
  T  = A + B                 on GPSIMD            (12F elems)
  PV = 2*T_even + T_odd      STT on DVE           (6F, pair values 0..6)
  VAL_k = PV_k + c_in        ONE prefix scan over pairs in reversed order
                             (LSB pair first):  state' = (G4 is_le state) + PV
                             G4 = 4.0, 10.0 at each row's LSB pair (resets
                             the carry at row boundaries). VAL in 0..7.
  sg4 = Sign(VAL - 3.5)      ACT   (= 2*c_out-1 per pair)
  R2  = VAL - 2*sg4          STT   (= (VAL mod 4) + 2, in 2..5)
  sg2 = Sign(R2 - 3.5)       ACT   (= 2*s_high-1)
  s_low  = R2 - sg2 - 3      STT -> odd output columns
  s_high = (sg2 + 1)/2       ACT copy -> even output columns
  carry  = (sg4|pair0 + 1)/2 ACT copy
All DMAs are plain HWDGE at line rate (loads on sync, stores on scalar).
"""
import numpy as np
import concourse.tile as tile
from concourse import bacc, mybir
from concourse.bass_utils import run_bass_kernel_spmd

N_BITS = 12
NPAIR = N_BITS // 2
BATCH = 4_194_304
N_CORES = 8
SHARD = BATCH // N_CORES        # 524288 rows per core
P = 128                         # SBUF partitions
F = 256                         # rows per partition per tile
ROWS_PER_TILE = P * F           # 32768
TILES = SHARD // ROWS_PER_TILE  # 16
W = N_BITS * F                  # 12F: full-width free elems per partition
WP = NPAIR * F                  # 6F: pair-domain free elems per partition


def _build():
    nc = bacc.Bacc("TRN2", target_bir_lowering=False, debug=False,
                   num_devices=N_CORES)
    A = nc.dram_tensor("A", [SHARD, N_BITS], mybir.dt.float32, kind="ExternalInput")
    B = nc.dram_tensor("B", [SHARD, N_BITS], mybir.dt.float32, kind="ExternalInput")
    S = nc.dram_tensor("S", [SHARD, N_BITS], mybir.dt.float32, kind="ExternalOutput")
    C = nc.dram_tensor("C", [SHARD, 1], mybir.dt.float32, kind="ExternalOutput")

    with tile.TileContext(nc) as tc:
        with tc.tile_pool(name="const", bufs=1) as constp, \
             tc.tile_pool(name="work", bufs=3) as work:
            # Pair-scan gate/threshold: 4.0 everywhere, 10.0 at pair 5 (LSB).
            G4 = constp.tile([P, WP], mybir.dt.float32)
            nc.vector.memset(G4[:], 4.0)
            G4v = G4[:].rearrange("p (f k) -> p f k", k=NPAIR)
            nc.vector.memset(G4v[:, :, NPAIR - 1 : NPAIR], 10.0)
            bneg = constp.tile([P, 1], mybir.dt.float32)
            nc.vector.memset(bneg[:], -3.5)

            for t in range(TILES):
                r0, r1 = t * ROWS_PER_TILE, (t + 1) * ROWS_PER_TILE
                Av = A[r0:r1, :].rearrange("(p f) b -> p (f b)", p=P)
                Bv = B[r0:r1, :].rearrange("(p f) b -> p (f b)", p=P)
                Sv = S[r0:r1, :].rearrange("(p f) b -> p (f b)", p=P)
                Cv = C[r0:r1, :].rearrange("(p f) b -> p (f b)", p=P)

                XA = work.tile([P, W], mybir.dt.float32)   # A -> T
                XB = work.tile([P, W], mybir.dt.float32)   # B -> S
                PV = work.tile([P, WP], mybir.dt.float32)  # pair values -> R2
                VA = work.tile([P, WP], mybir.dt.float32)  # scan out VAL
                s4 = work.tile([P, WP], mybir.dt.float32)  # sign(VAL-3.5)
                s2 = work.tile([P, WP], mybir.dt.float32)  # sign(R2-3.5)
                co = work.tile([P, F], mybir.dt.float32)   # final carry per row

                nc.sync.dma_start(out=XA[:], in_=Av)
                nc.sync.dma_start(out=XB[:], in_=Bv)

                nc.gpsimd.tensor_tensor(out=XA[:], in0=XA[:], in1=XB[:],
                                        op=mybir.AluOpType.add)

                T3 = XA[:].rearrange("p (f b) -> p f b", b=N_BITS)
                PV3 = PV[:].rearrange("p (f k) -> p f k", k=NPAIR)
                # PV = 2*T_even + T_odd
                nc.vector.scalar_tensor_tensor(
                    out=PV3[:, :, :], in0=T3[:, :, 0::2], scalar=2.0,
                    in1=T3[:, :, 1::2],
                    op0=mybir.AluOpType.mult, op1=mybir.AluOpType.add,
                )

                # radix-4 carry scan over pairs, reversed (LSB pair first).
                # WP = 1536 <= 2040, so a single instruction per tile.
                nc.vector.tensor_tensor_scan(
                    out=VA[:][:, ::-1], data0=G4[:][:, ::-1],
                    data1=PV[:][:, ::-1], initial=0.0,
                    op0=mybir.AluOpType.is_le, op1=mybir.AluOpType.add,
                )

                # sg4 = sign(VAL - 3.5)
                nc.scalar.activation(s4[:], VA[:],
                                     mybir.ActivationFunctionType.Sign,
                                     bias=bneg[:], scale=1.0)
                # R2 = VAL - 2*sg4  (= (VAL mod 4) + 2), into PV (dead)
                nc.vector.scalar_tensor_tensor(
                    out=PV[:], in0=s4[:], scalar=-2.0, in1=VA[:],
                    op0=mybir.AluOpType.mult, op1=mybir.AluOpType.add,
                )
                # sg2 = sign(R2 - 3.5)
                nc.scalar.activation(s2[:], PV[:],
                                     mybir.ActivationFunctionType.Sign,
                                     bias=bneg[:], scale=1.0)

                S3 = XB[:].rearrange("p (f b) -> p f b", b=N_BITS)
                # s_low = (R2 - 3) - sg2 -> odd columns
                nc.vector.scalar_tensor_tensor(
                    out=S3[:, :, 1::2], in0=PV3[:, :, :], scalar=-3.0,
                    in1=s2[:].rearrange("p (f k) -> p f k", k=NPAIR),
                    op0=mybir.AluOpType.add, op1=mybir.AluOpType.subtract,
                )
                # s_high = (sg2 + 1)/2 -> even columns
                nc.scalar.activation(S3[:, :, 0::2],
                                     s2[:].rearrange("p (f k) -> p f k", k=NPAIR),
                                     mybir.ActivationFunctionType.Copy,
                                     bias=0.5, scale=0.5)
                # carry = (sg4|pair0 + 1)/2
                s4v = s4[:].rearrange("p (f k) -> p f k", k=NPAIR)
                nc.scalar.activation(co[:], s4v[:, :, 0:1],
                                     mybir.ActivationFunctionType.Copy,
                                     bias=0.5, scale=0.5)

                nc.scalar.dma_start(out=Sv, in_=XB[:])
                nc.scalar.dma_start(out=Cv, in_=co[:])
    nc.compile()
    return nc


_NC = None


def kernel(A: np.ndarray, B: np.ndarray):
    global _NC
    if _NC is None:
        _NC = _build()
    A = np.ascontiguousarray(A, dtype=np.float32)
    B = np.ascontiguousarray(B, dtype=np.float32)
    in_maps = [
        {"A": A[i * SHARD : (i + 1) * SHARD], "B": B[i * SHARD : (i + 1) * SHARD]}
        for i in range(N_CORES)
    ]
    res = run_bass_kernel_spmd(_NC, in_maps, core_ids=list(range(N_CORES)))
    S = np.concatenate([r["S"] for r in res.results], axis=0)
    C = np.concatenate([r["C"] for r in res.results], axis=0)
    return S, C
